# revision 6
# baseline (speedup 1.0000x reference)
"""Trainium2 Bass kernel for GatedGraphXBias (gnn_message_passing).

Reference math per iteration (T=2048 notes, E=12 edge types, H=64):
    act[e]  = edge[e].T @ h                      # [T, H]
    a       = sum_e (act[e] + ba[e]) @ W[e] + bw # [T, 3H] -> az|ar|ah
    a      += x @ Win                            # hoisted input projection
    z       = sigmoid(az + h @ Uz)
    r       = sigmoid(ar + h @ Ur)
    h~      = tanh(ah + (r*h) @ Uh)
    h       = (1-z)*h + z*h~

Sharding: sequence-parallel over the note dim T across 8 cores (256 notes
each).  Each core keeps its edge shard [12, 2048, 256] resident in SBUF as
fp16 (12.6 MiB) and the full h replicated as fp16 matmul weights; per
iteration the updated h shard is AllGather'd (fp16, 32 KiB payload).

The big matmuls (edge contraction mm1 and the per-edge-type W projection
mm2) run in fp16 — full 16-bit PE rate, 2x over fp32r, final rel err
~2e-3 (validated against an fp32 numpy sim).  Gate math stays fp32.

The full-h weight copy uses a permuted source layout (source note
s = f(partition, chunk) chosen so the per-iteration h reload from the
AllGather output is 128 contiguous 2 KiB descriptors instead of 2048
strided 256 B ones).  mm2 for pair p-1 is emitted after mm1 of pair p so
the PE never stalls on the PSUM->SBUF act copy.
"""

import sys

sys.path.insert(0, "/opt/trn_rl_repo")

import numpy as np
import concourse.bass as bass
import concourse.mybir as mybir
import concourse.tile as tile
from concourse.bass_utils import run_bass_kernel_spmd
from concourse.masks import make_identity
from concourse.vector_clock import ScopedClock

E, T, H, IN = 12, 2048, 64, 128
M = 8  # cores
TL = T // M  # 256 local notes per core
NCH = T // 128  # 16 contraction chunks of 128 source notes
NPAIR = E // 2  # edge types processed two at a time (n=512 matmuls)
F32 = mybir.dt.float32
F32R = mybir.dt.float32r
F16 = mybir.dt.float16
SIG = mybir.ActivationFunctionType.Sigmoid
TANH = mybir.ActivationFunctionType.Tanh


def _source_perm() -> np.ndarray:
    """s_of[c_idx, p] = global source note held at (chunk c_idx, partition p).

    Chosen so that the h reload from the AllGather output [8*128, 128]
    (row r = rank*128 + p2 holding h rows rank*256 + {0,1}*128 + p2 as two
    64-col blocks) is contiguous per SBUF partition: partition p reads
    DRAM rows p*8 .. p*8+7 (2 KiB).  Chunk c_idx = k*2 + ch maps to
    ag-row r = p*8 + k, giving s = (r//128)*256 + ch*128 + (r%128).
    """
    p = np.arange(128)[None, :]
    c_idx = np.arange(NCH)[:, None]
    k, ch = c_idx // 2, c_idx % 2
    r = p * 8 + k
    return (r // 128) * 256 + ch * 128 + (r % 128)


_S_OF = _source_perm()  # [16, 128]


class SplitDrainTileContext(tile.TileContext):
    """TileContext that limits every instruction to a single sync wait.

    This walrus build rejects >1 sync wait command on an instruction
    (setupSyncWait: "Too many sync wait commands"), so extra waits are
    peeled onto standalone same-engine NoOps emitted just before the
    instruction — semantically identical (the engine stream waits
    sequentially at the same program point)."""

    def _commit_instruction(self, inst, lazy_reg_writes: bool = True):
        si = getattr(inst, "sync_info", None)
        if si is not None and len(si.on_wait) > 1:
            waits = list(si.on_wait)
            inst.sync_info = mybir.SyncInfo(
                on_wait=[waits[-1]], on_update=list(si.on_update)
            )
            for w in waits[:-1]:
                nop = mybir.InstNoOp(
                    name=f"splitwait-{self.nc.next_id()}",
                    sync_info=mybir.SyncInfo(on_wait=[w], on_update=[]),
                    bass_nofuse=True,
                    engine=inst.engine,
                )
                super()._commit_instruction(nop, lazy_reg_writes=False)
        super()._commit_instruction(inst, lazy_reg_writes)

    def _drain_and_barrier(self, tick_clock, wait_clock):
        drain_inst = self.nc.sync.drain()
        wait_clock.add_sem_waits(
            drain_inst.ins, ScopedClock({None: tick_clock.global_clock})
        )
        si = drain_inst.ins.sync_info
        waits = list(si.on_wait) if si is not None else []
        upds = list(si.on_update) if si is not None else []
        if len(waits) > 1:
            drain_inst.ins.sync_info = mybir.SyncInfo(on_wait=waits[:1], on_update=upds)
            for w in waits[1:]:
                nop = self.nc.sync.nop(nofuse=True, hint="split_drain_waits")
                nop.ins.sync_info = mybir.SyncInfo(on_wait=[w], on_update=[])

        self.nc.all_engine_barrier()
        assert self.sems is not None
        popped = self.nc._tile_sem_poison_stack.pop()
        assert popped is self._sem_poison
        self.nc.clear_and_free_semaphores(list(self.sems.allocated().values()))
        self.nc.all_engine_barrier()


def build(iteration: int, reps: int = 1, ablate: frozenset = frozenset()) -> bass.Bass:
    # dynamic_dma_scratch_size: SWDGE (gpsimd) descriptor carveout in SBUF.
    # All our DMAs go through the SP/ACT hardware DGE queues, so shrink it
    # from the default 16 KiB/partition.
    nc = bass.Bass(
        "TRN2",
        target_bir_lowering=False,
        debug=False,
        num_devices=M,
        dynamic_dma_scratch_size=2048,
    )

    # Per-core inputs (host pre-arranged into DMA-friendly layouts):
    #   edge_in : [T, E*TL] f16  row = c_idx*128+p (permuted source note
    #             _S_OF[c_idx, p]), col = e*TL + t_local
    #   hid_in  : [8*128, 128] f16  full initial hidden in AG-output layout
    #   hT0_in  : [H, TL]    local initial hidden, transposed (f32)
    #   xT_in   : [IN, TL]   local input features, transposed (f32)
    #   w_in    : [H, E*3H] f16  W[e] as lhsT blocks, col = e*192 + k
    #   uzr_in  : [H, 2H]    f32r
    #   uh_in   : [H, H]     f32r
    #   win_in  : [IN, 3H]   f32r
    #   bz/br/bh: [H, 1]     folded biases (bw + sum_e ba[e] @ W[e])
    edge_in = nc.declare_dram_parameter("edge_in", [T, E * TL], F16, isOutput=False)
    hid_in = nc.declare_dram_parameter("hid_in", [M * 128, 128], F16, isOutput=False)
    hT0_in = nc.declare_dram_parameter("hT0_in", [H, TL], F32R, isOutput=False)
    xT_in = nc.declare_dram_parameter("xT_in", [IN, TL], F32R, isOutput=False)
    w_in = nc.declare_dram_parameter("w_in", [H, E * 3 * H], F16, isOutput=False)
    uzr_in = nc.declare_dram_parameter("uzr_in", [H, 2 * H], F32R, isOutput=False)
    uh_in = nc.declare_dram_parameter("uh_in", [H, H], F32R, isOutput=False)
    win_in = nc.declare_dram_parameter("win_in", [IN, 3 * H], F32R, isOutput=False)
    bz_in = nc.declare_dram_parameter("bz_in", [H, 1], F32, isOutput=False)
    br_in = nc.declare_dram_parameter("br_in", [H, 1], F32, isOutput=False)
    bh_in = nc.declare_dram_parameter("bh_in", [H, 1], F32, isOutput=False)
    h_out = nc.declare_dram_parameter("h_out", [TL, H], F32, isOutput=True)

    with SplitDrainTileContext(nc) as tc:
        with (
            tc.tile_pool(name="edge", bufs=1) as edge_pool,
            tc.tile_pool(name="const", bufs=1) as cpool,
            tc.tile_pool(name="work", bufs=1) as wpool,
            tc.tile_pool(name="psum", bufs=1, space="PSUM") as ppool,
            tc.tile_pool(name="dram", bufs=2, space="DRAM") as dpool,
        ):
            # ---- constants / weights (loaded once) ----
            w_sb = cpool.tile([H, E * 3 * H], F16)
            nc.sync.dma_start(out=w_sb[:], in_=w_in[:])
            uzr_sb = cpool.tile([H, 2 * H], F32R)
            nc.sync.dma_start(out=uzr_sb[:], in_=uzr_in[:])
            uh_sb = cpool.tile([H, H], F32R)
            nc.sync.dma_start(out=uh_sb[:], in_=uh_in[:])
            win_sb = cpool.tile([IN, 3 * H], F32R)
            nc.sync.dma_start(out=win_sb[:], in_=win_in[:])
            xT_sb = cpool.tile([IN, TL], F32R)
            nc.sync.dma_start(out=xT_sb[:], in_=xT_in[:])
            bz_sb = cpool.tile([H, 1], F32)
            nc.sync.dma_start(out=bz_sb[:], in_=bz_in[:])
            br_sb = cpool.tile([H, 1], F32)
            nc.sync.dma_start(out=br_sb[:], in_=br_in[:])
            bh_sb = cpool.tile([H, 1], F32)
            nc.sync.dma_start(out=bh_sb[:], in_=bh_in[:])
            id64 = cpool.tile([H, H], F32)
            make_identity(nc, id64[:])

            for rep in range(reps):
                # ---- resident edge shard: 16 chunk tiles [128, E*TL] f16 ----
                edge_sb = []
                for c in range(NCH):
                    et = edge_pool.tile(
                        [128, E * TL], F16, name=f"edge_c{c}", tag=f"edge_c{c}",
                        bufs=2,
                    )
                    nc.sync.dma_start(
                        out=et[:], in_=edge_in[c * 128 : (c + 1) * 128, :]
                    )
                    edge_sb.append(et)

                # ---- h state: full h as f16 weights + local hT (f32) ----
                # h_sb[p, c_idx*64 + j] = h[_S_OF[c_idx, p], j]; the load is
                # 8 contiguous 256 B blocks per partition (2 KiB).
                h_sb = wpool.tile([128, NCH * H], F16, name="h", tag="h", bufs=2)
                nc.sync.dma_start(
                    out=h_sb[:].rearrange("p (k x) -> p k x", k=8),
                    in_=hid_in[:].rearrange("(p k) x -> p k x", p=128),
                )
                hT_sb = wpool.tile([H, TL], F32R, name="hT", tag="hT", bufs=2)
                nc.sync.dma_start(out=hT_sb[:], in_=hT0_in[:])

                for it in range(iteration):
                    last = it == iteration - 1

                    # Three PSUM accumulation groups (all [H, TL] base
                    # partition 0): az, ar, ah.  Each starts with the folded
                    # input projection, absorbs the U-gate matmul, then the
                    # 12 per-edge-type W matmuls.
                    az_ps = ppool.tile([H, TL], F32, tag="az")
                    ar_ps = ppool.tile([H, TL], F32, tag="ar")
                    ah_ps = ppool.tile([H, TL], F32, tag="ah")
                    for g, ps in enumerate((az_ps, ar_ps, ah_ps)):
                        nc.tensor.matmul(
                            ps[:],
                            lhsT=win_sb[:, g * H : (g + 1) * H],
                            rhs=xT_sb[:],
                            start=True,
                            stop=False,
                            skip_group_check=True,
                        )
                    for g, ps in enumerate((az_ps, ar_ps)):
                        nc.tensor.matmul(
                            ps[:],
                            lhsT=uzr_sb[:, g * H : (g + 1) * H],
                            rhs=hT_sb[:],
                            start=False,
                            stop="mm2" in ablate or "mm1" in ablate,
                            skip_group_check=True,
                        )

                    # -- mm1: actT per e-pair; mm2 for pair p-1 is emitted
                    # after mm1 of pair p so the PE never waits on the
                    # PSUM->SBUF act copy --
                    def emit_mm1(pair):
                        act_ps = ppool.tile([H, 2 * TL], F32, tag="actT", bufs=3)
                        for c in range(NCH):
                            nc.tensor.matmul(
                                act_ps[:],
                                lhsT=h_sb[:, c * H : (c + 1) * H],
                                rhs=edge_sb[c][
                                    :, pair * 2 * TL : (pair + 1) * 2 * TL
                                ],
                                start=(c == 0),
                                stop=(c == NCH - 1),
                                skip_group_check=True,
                            )
                        act_sb = wpool.tile(
                            [H, 2 * TL], F16, name="act", tag="act", bufs=2
                        )
                        nc.vector.tensor_copy(act_sb[:], act_ps[:])
                        return act_sb

                    def emit_mm2(pair, act_sb):
                        if "mm2" in ablate:
                            return
                        for k in range(2):
                            e = pair * 2 + k
                            for g, ps in enumerate((az_ps, ar_ps, ah_ps)):
                                nc.tensor.matmul(
                                    ps[:],
                                    lhsT=w_sb[
                                        :,
                                        e * 3 * H + g * H : e * 3 * H + (g + 1) * H,
                                    ],
                                    rhs=act_sb[:, k * TL : (k + 1) * TL],
                                    start=False,
                                    stop=(e == E - 1 and g != 2),
                                    skip_group_check=True,
                                )

                    if "mm1" not in ablate:
                        prev = None
                        for pair in range(NPAIR):
                            act_sb = emit_mm1(pair)
                            if prev is not None:
                                emit_mm2(pair - 1, prev)
                            prev = act_sb
                        emit_mm2(NPAIR - 1, prev)

                    # -- gates --
                    z_sb = wpool.tile([H, TL], F32, tag="z")
                    nc.scalar.activation(z_sb[:], az_ps[:], SIG, bias=bz_sb[:])
                    r_sb = wpool.tile([H, TL], F32, tag="r")
                    nc.scalar.activation(r_sb[:], ar_ps[:], SIG, bias=br_sb[:])
                    rh_sb = wpool.tile([H, TL], F32R, tag="rh")
                    nc.vector.tensor_mul(rh_sb[:], r_sb[:], hT_sb[:])
                    nc.tensor.matmul(
                        ah_ps[:],
                        lhsT=uh_sb[:],
                        rhs=rh_sb[:],
                        start=False,
                        stop=True,
                        skip_group_check=True,
                    )
                    ht_sb = wpool.tile([H, TL], F32, tag="ht")
                    nc.scalar.activation(ht_sb[:], ah_ps[:], TANH, bias=bh_sb[:])

                    d_sb = wpool.tile([H, TL], F32, tag="d")
                    nc.vector.tensor_sub(d_sb[:], ht_sb[:], hT_sb[:])
                    zd_sb = wpool.tile([H, TL], F32, tag="zd")
                    nc.vector.tensor_mul(zd_sb[:], z_sb[:], d_sb[:])
                    hnewT_sb = wpool.tile([H, TL], F32R, tag="hT", bufs=2)
                    nc.vector.tensor_add(hnewT_sb[:], zd_sb[:], hT_sb[:])

                    # -- transpose hnewT -> [TL-chunked, H] payload --
                    if last:
                        hnew_sb = wpool.tile([128, 2 * H], F32, tag="hnew32")
                    else:
                        hnew_sb = wpool.tile([128, 2 * H], F16, tag="hnew16")
                    for half in range(2):
                        tr_ps = ppool.tile([128, H], F32, tag="tr")
                        nc.tensor.transpose(
                            tr_ps[:],
                            hnewT_sb[:, half * 128 : (half + 1) * 128].bitcast(F32),
                            id64[:],
                        )
                        nc.vector.tensor_copy(
                            hnew_sb[:, half * H : (half + 1) * H], tr_ps[:]
                        )

                    if last:
                        nc.sync.dma_start(
                            out=h_out[:].rearrange("(c p) j -> p c j", p=128),
                            in_=hnew_sb[:].rearrange("p (c j) -> p c j", c=2),
                        )
                    else:
                        if "coll" not in ablate:
                            # payload [128, 128] f16: row p = local notes
                            # {p, 128+p} as two 64-col blocks -> one fully
                            # contiguous 256 B descriptor per partition.
                            ag_in = dpool.tile([128, 128], F16, tag="ag_in")
                            nc.sync.dma_start(out=ag_in[:], in_=hnew_sb[:])
                            ag_out = dpool.tile(
                                [M * 128, 128], F16, tag="ag_out",
                                addr_space="Shared",
                            )
                            nc.gpsimd.collective_compute(
                                "AllGather",
                                mybir.AluOpType.bypass,
                                replica_groups=[list(range(M))],
                                ins=[ag_in[:]],
                                outs=[ag_out[:]],
                            )
                            gather_src = ag_out
                        else:
                            gather_src = hid_in
                        if "hreload" not in ablate:
                            h_sb = wpool.tile([128, NCH * H], F16, tag="h", bufs=2)
                            nc.sync.dma_start(
                                out=h_sb[:].rearrange("p (k x) -> p k x", k=8),
                                in_=gather_src[:].rearrange(
                                    "(p k) x -> p k x", p=128
                                ),
                            )
                        hT_sb = hnewT_sb

    return nc


def _host_prep(input, hidden, edge_matrix, ba, wz_wr_wh, uz_ur, uh, input_wzrh, bw):
    """Pre-arrange full inputs into the per-core DMA layouts."""
    x = np.asarray(input, np.float32)[0]  # [T, IN]
    h0 = np.ascontiguousarray(np.asarray(hidden, np.float32)[0])  # [T, H]
    edge = np.asarray(edge_matrix, np.float32)  # [E, T, T]
    ba = np.asarray(ba, np.float32)
    W = np.asarray(wz_wr_wh, np.float32)  # [E, H, 3H]
    uzr = np.ascontiguousarray(np.asarray(uz_ur, np.float32))
    uh_ = np.ascontiguousarray(np.asarray(uh, np.float32))
    win = np.ascontiguousarray(np.asarray(input_wzrh, np.float32))
    bw = np.asarray(bw, np.float32)

    # folded bias: bw + sum_e ba[e] @ W[e]
    btot = bw + np.einsum("eh,ehk->k", ba, W)  # [3H]
    bz = np.ascontiguousarray(btot[:H].reshape(H, 1))
    br = np.ascontiguousarray(btot[H : 2 * H].reshape(H, 1))
    bh = np.ascontiguousarray(btot[2 * H :].reshape(H, 1))

    # edge shards with the permuted source layout (fp16):
    #   esh[m][c_idx*128 + p, e*TL + tl] = edge[e, _S_OF[c_idx, p], m*TL + tl]
    edge_perm = edge[:, _S_OF.reshape(-1), :]  # [E, 2048(perm), T]
    esh = np.ascontiguousarray(
        edge_perm.reshape(E, T, M, TL).transpose(2, 1, 0, 3), dtype=np.float16
    ).reshape(M, T, E * TL)

    # initial h in AG-output layout [8*128, 128] f16:
    #   hid[r, ch*64 + j] = h0[(r//128)*256 + ch*128 + (r%128), j]
    hid = np.ascontiguousarray(
        h0.reshape(M, 2, 128, H).transpose(0, 2, 1, 3).reshape(M * 128, 128),
        dtype=np.float16,
    )

    w_flat = np.ascontiguousarray(
        W.transpose(1, 0, 2), dtype=np.float16
    ).reshape(H, E * 3 * H)

    in_maps = []
    for m in range(M):
        xT = np.ascontiguousarray(x[m * TL : (m + 1) * TL, :].T)
        hT0 = np.ascontiguousarray(h0[m * TL : (m + 1) * TL, :].T)
        in_maps.append(
            {
                "edge_in": esh[m],
                "hid_in": hid,
                "hT0_in": hT0,
                "xT_in": xT,
                "w_in": w_flat,
                "uzr_in": uzr,
                "uh_in": uh_,
                "win_in": win,
                "bz_in": bz,
                "br_in": br,
                "bh_in": bh,
            }
        )
    return in_maps


_NC_CACHE: dict = {}


def _get_nc(iteration: int, reps: int = 1, ablate: frozenset = frozenset()) -> bass.Bass:
    key = (iteration, reps, ablate)
    if key not in _NC_CACHE:
        _NC_CACHE[key] = build(iteration, reps=reps, ablate=ablate)
    return _NC_CACHE[key]


def kernel(
    input,
    hidden,
    edge_matrix,
    ba,
    wz_wr_wh,
    uz_ur,
    uh,
    input_wzrh,
    bw,
    iteration,
):
    iteration = int(iteration)
    if iteration <= 0:
        return np.asarray(hidden, np.float32).copy()

    nc = _get_nc(iteration)
    in_maps = _host_prep(
        input, hidden, edge_matrix, ba, wz_wr_wh, uz_ur, uh, input_wzrh, bw
    )
    res = run_bass_kernel_spmd(nc, in_maps, list(range(M)))
    out = np.concatenate([res.results[m]["h_out"] for m in range(M)], axis=0)
    return out[None]


# revision 7
# speedup vs baseline: 6.1881x; 6.1881x over previous
"""Trainium2 Bass kernel for GatedGraphXBias (gnn_message_passing).

Reference math per iteration (T=2048 notes, E=12 edge types, H=64):
    act[e]  = edge[e].T @ h                      # [T, H]
    a       = sum_e (act[e] + ba[e]) @ W[e] + bw # [T, 3H] -> az|ar|ah
    a      += x @ Win                            # hoisted input projection
    z       = sigmoid(az + h @ Uz)
    r       = sigmoid(ar + h @ Ur)
    h~      = tanh(ah + (r*h) @ Uh)
    h       = (1-z)*h + z*h~

Sharding: sequence-parallel over the note dim T across 8 cores (256 notes
each).  Each core keeps its edge shard [12, 2048, 256] resident in SBUF as
fp16 (12.6 MiB) and the full h replicated as fp16 matmul weights; per
iteration the updated h shard is AllGather'd (fp16, 32 KiB payload).

The big matmuls (edge contraction mm1 and the per-edge-type W projection
mm2) run in fp16 — full 16-bit PE rate, 2x over fp32r, final rel err
~2e-3 (validated against an fp32 numpy sim).  Gate math stays fp32.

The full-h weight copy uses a permuted source layout (source note
s = f(partition, chunk) chosen so the per-iteration h reload from the
AllGather output is 128 contiguous 2 KiB descriptors instead of 2048
strided 256 B ones).  mm2 for pair p-1 is emitted after mm1 of pair p so
the PE never stalls on the PSUM->SBUF act copy.
"""

import sys

sys.path.insert(0, "/opt/trn_rl_repo")

import numpy as np
import concourse.bass as bass
import concourse.mybir as mybir
import concourse.tile as tile
from concourse.bass_utils import run_bass_kernel_spmd
from concourse.masks import make_identity
from concourse.vector_clock import ScopedClock

E, T, H, IN = 12, 2048, 64, 128
M = 8  # cores
TL = T // M  # 256 local notes per core
NCH = T // 128  # 16 contraction chunks of 128 source notes
NPAIR = E // 2  # edge types processed two at a time (n=512 matmuls)
F32 = mybir.dt.float32
F32R = mybir.dt.float32r
F16 = mybir.dt.float16
SIG = mybir.ActivationFunctionType.Sigmoid
TANH = mybir.ActivationFunctionType.Tanh


def _source_perm() -> np.ndarray:
    """s_of[c_idx, p] = global source note held at (chunk c_idx, partition p).

    Chosen so that the h reload from the AllGather output [8*128, 128]
    (row r = rank*128 + p2 holding h rows rank*256 + {0,1}*128 + p2 as two
    64-col blocks) is contiguous per SBUF partition: partition p reads
    DRAM rows p*8 .. p*8+7 (2 KiB).  Chunk c_idx = k*2 + ch maps to
    ag-row r = p*8 + k, giving s = (r//128)*256 + ch*128 + (r%128).
    """
    p = np.arange(128)[None, :]
    c_idx = np.arange(NCH)[:, None]
    k, ch = c_idx // 2, c_idx % 2
    r = p * 8 + k
    return (r // 128) * 256 + ch * 128 + (r % 128)


_S_OF = _source_perm()  # [16, 128]


class SplitDrainTileContext(tile.TileContext):
    """TileContext that limits every instruction to a single sync wait.

    This walrus build rejects >1 sync wait command on an instruction
    (setupSyncWait: "Too many sync wait commands"), so extra waits are
    peeled onto standalone same-engine NoOps emitted just before the
    instruction — semantically identical (the engine stream waits
    sequentially at the same program point)."""

    def _commit_instruction(self, inst, lazy_reg_writes: bool = True):
        si = getattr(inst, "sync_info", None)
        if si is not None and len(si.on_wait) > 1:
            waits = list(si.on_wait)
            inst.sync_info = mybir.SyncInfo(
                on_wait=[waits[-1]], on_update=list(si.on_update)
            )
            for w in waits[:-1]:
                nop = mybir.InstNoOp(
                    name=f"splitwait-{self.nc.next_id()}",
                    sync_info=mybir.SyncInfo(on_wait=[w], on_update=[]),
                    bass_nofuse=True,
                    engine=inst.engine,
                )
                super()._commit_instruction(nop, lazy_reg_writes=False)
        super()._commit_instruction(inst, lazy_reg_writes)

    def _drain_and_barrier(self, tick_clock, wait_clock):
        drain_inst = self.nc.sync.drain()
        wait_clock.add_sem_waits(
            drain_inst.ins, ScopedClock({None: tick_clock.global_clock})
        )
        si = drain_inst.ins.sync_info
        waits = list(si.on_wait) if si is not None else []
        upds = list(si.on_update) if si is not None else []
        if len(waits) > 1:
            drain_inst.ins.sync_info = mybir.SyncInfo(on_wait=waits[:1], on_update=upds)
            for w in waits[1:]:
                nop = self.nc.sync.nop(nofuse=True, hint="split_drain_waits")
                nop.ins.sync_info = mybir.SyncInfo(on_wait=[w], on_update=[])

        self.nc.all_engine_barrier()
        assert self.sems is not None
        popped = self.nc._tile_sem_poison_stack.pop()
        assert popped is self._sem_poison
        self.nc.clear_and_free_semaphores(list(self.sems.allocated().values()))
        self.nc.all_engine_barrier()


def build(iteration: int, reps: int = 1, ablate: frozenset = frozenset()) -> bass.Bass:
    # dynamic_dma_scratch_size: SWDGE (gpsimd) descriptor carveout in SBUF.
    # All our DMAs go through the SP/ACT hardware DGE queues, so shrink it
    # from the default 16 KiB/partition.
    nc = bass.Bass(
        "TRN2",
        target_bir_lowering=False,
        debug=False,
        num_devices=M,
        dynamic_dma_scratch_size=2048,
    )

    # Per-core inputs (host pre-arranged into DMA-friendly layouts):
    #   edge_in : [T, E*TL] f16  row = c_idx*128+p (permuted source note
    #             _S_OF[c_idx, p]), col = e*TL + t_local
    #   hid_in  : [8*128, 128] f16  full initial hidden in AG-output layout
    #   hT0_in  : [H, TL]    local initial hidden, transposed (f32)
    #   xT_in   : [IN, TL]   local input features, transposed (f32)
    #   w_in    : [H, E*3H] f16  W[e] as lhsT blocks, col = e*192 + k
    #   uzr_in  : [H, 2H]    f32r
    #   uh_in   : [H, H]     f32r
    #   win_in  : [IN, 3H]   f32r
    #   bz/br/bh: [H, 1]     folded biases (bw + sum_e ba[e] @ W[e])
    edge_in = nc.declare_dram_parameter("edge_in", [T, E * TL], F16, isOutput=False)
    hid_in = nc.declare_dram_parameter("hid_in", [M * 128, 128], F16, isOutput=False)
    hT0_in = nc.declare_dram_parameter("hT0_in", [H, TL], F32R, isOutput=False)
    xT_in = nc.declare_dram_parameter("xT_in", [IN, TL], F32R, isOutput=False)
    w_in = nc.declare_dram_parameter("w_in", [H, E * 3 * H], F16, isOutput=False)
    uzr_in = nc.declare_dram_parameter("uzr_in", [H, 2 * H], F32R, isOutput=False)
    uh_in = nc.declare_dram_parameter("uh_in", [H, H], F32R, isOutput=False)
    win_in = nc.declare_dram_parameter("win_in", [IN, 3 * H], F32R, isOutput=False)
    bz_in = nc.declare_dram_parameter("bz_in", [H, 1], F32, isOutput=False)
    br_in = nc.declare_dram_parameter("br_in", [H, 1], F32, isOutput=False)
    bh_in = nc.declare_dram_parameter("bh_in", [H, 1], F32, isOutput=False)
    h_out = nc.declare_dram_parameter("h_out", [TL, H], F32, isOutput=True)

    with SplitDrainTileContext(nc) as tc:
        with (
            tc.tile_pool(name="edge", bufs=1) as edge_pool,
            tc.tile_pool(name="const", bufs=1) as cpool,
            tc.tile_pool(name="work", bufs=1) as wpool,
            tc.tile_pool(name="psum", bufs=1, space="PSUM") as ppool,
            tc.tile_pool(name="dram", bufs=4, space="DRAM") as dpool,
        ):
            # ---- constants / weights (loaded once) ----
            w_sb = cpool.tile([H, E * 3 * H], F16)
            nc.sync.dma_start(out=w_sb[:], in_=w_in[:])
            uzr_sb = cpool.tile([H, 2 * H], F32R)
            nc.sync.dma_start(out=uzr_sb[:], in_=uzr_in[:])
            uh_sb = cpool.tile([H, H], F32R)
            nc.sync.dma_start(out=uh_sb[:], in_=uh_in[:])
            win_sb = cpool.tile([IN, 3 * H], F32R)
            nc.sync.dma_start(out=win_sb[:], in_=win_in[:])
            xT_sb = cpool.tile([IN, TL], F32R)
            nc.sync.dma_start(out=xT_sb[:], in_=xT_in[:])
            bz_sb = cpool.tile([H, 1], F32)
            nc.sync.dma_start(out=bz_sb[:], in_=bz_in[:])
            br_sb = cpool.tile([H, 1], F32)
            nc.sync.dma_start(out=br_sb[:], in_=br_in[:])
            bh_sb = cpool.tile([H, 1], F32)
            nc.sync.dma_start(out=bh_sb[:], in_=bh_in[:])
            id64 = cpool.tile([H, H], F32)
            make_identity(nc, id64[:])

            for rep in range(reps):
                # ---- resident edge shard: 16 chunk tiles [128, E*TL] f16 ----
                edge_sb = []
                for c in range(NCH):
                    et = edge_pool.tile(
                        [128, E * TL], F16, name=f"edge_c{c}", tag=f"edge_c{c}",
                        bufs=2,
                    )
                    nc.sync.dma_start(
                        out=et[:], in_=edge_in[c * 128 : (c + 1) * 128, :]
                    )
                    edge_sb.append(et)

                # ---- h state: full h as f16 weights + local hT (f32) ----
                # h_sb[p, c_idx*64 + j] = h[_S_OF[c_idx, p], j]; the load is
                # 8 contiguous 256 B blocks per partition (2 KiB).
                h_sb = wpool.tile([128, NCH * H], F16, name="h", tag="h", bufs=2)
                nc.sync.dma_start(
                    out=h_sb[:].rearrange("p (k x) -> p k x", k=8),
                    in_=hid_in[:].rearrange("(p k) x -> p k x", p=128),
                )
                hT_sb = wpool.tile([H, TL], F32R, name="hT", tag="hT", bufs=2)
                nc.sync.dma_start(out=hT_sb[:], in_=hT0_in[:])

                for it in range(iteration):
                    last = it == iteration - 1

                    # Three PSUM accumulation groups (all [H, TL] base
                    # partition 0): az, ar, ah.  Each starts with the folded
                    # input projection, absorbs the U-gate matmul, then the
                    # 12 per-edge-type W matmuls.
                    az_ps = ppool.tile([H, TL], F32, tag="az")
                    ar_ps = ppool.tile([H, TL], F32, tag="ar")
                    ah_ps = ppool.tile([H, TL], F32, tag="ah")
                    for g, ps in enumerate((az_ps, ar_ps, ah_ps)):
                        nc.tensor.matmul(
                            ps[:],
                            lhsT=win_sb[:, g * H : (g + 1) * H],
                            rhs=xT_sb[:],
                            start=True,
                            stop=False,
                            skip_group_check=True,
                        )
                    for g, ps in enumerate((az_ps, ar_ps)):
                        nc.tensor.matmul(
                            ps[:],
                            lhsT=uzr_sb[:, g * H : (g + 1) * H],
                            rhs=hT_sb[:],
                            start=False,
                            stop="mm2" in ablate or "mm1" in ablate,
                            skip_group_check=True,
                        )

                    # -- mm1: actT per e-pair; mm2 for pair p-1 is emitted
                    # after mm1 of pair p so the PE never waits on the
                    # PSUM->SBUF act copy --
                    def emit_mm1(pair):
                        act_ps = ppool.tile([H, 2 * TL], F32, tag="actT", bufs=3)
                        for c in range(NCH):
                            nc.tensor.matmul(
                                act_ps[:],
                                lhsT=h_sb[:, c * H : (c + 1) * H],
                                rhs=edge_sb[c][
                                    :, pair * 2 * TL : (pair + 1) * 2 * TL
                                ],
                                start=(c == 0),
                                stop=(c == NCH - 1),
                                skip_group_check=True,
                            )
                        act_sb = wpool.tile(
                            [H, 2 * TL], F16, name="act", tag="act", bufs=2
                        )
                        nc.vector.tensor_copy(act_sb[:], act_ps[:])
                        return act_sb

                    def emit_mm2(pair, act_sb):
                        if "mm2" in ablate:
                            return
                        for k in range(2):
                            e = pair * 2 + k
                            for g, ps in enumerate((az_ps, ar_ps, ah_ps)):
                                nc.tensor.matmul(
                                    ps[:],
                                    lhsT=w_sb[
                                        :,
                                        e * 3 * H + g * H : e * 3 * H + (g + 1) * H,
                                    ],
                                    rhs=act_sb[:, k * TL : (k + 1) * TL],
                                    start=False,
                                    stop=(e == E - 1 and g != 2),
                                    skip_group_check=True,
                                )

                    if "mm1" not in ablate:
                        prev = None
                        for pair in range(NPAIR):
                            act_sb = emit_mm1(pair)
                            if prev is not None:
                                emit_mm2(pair - 1, prev)
                            prev = act_sb
                        emit_mm2(NPAIR - 1, prev)

                    # -- gates --
                    z_sb = wpool.tile([H, TL], F32, tag="z")
                    nc.scalar.activation(z_sb[:], az_ps[:], SIG, bias=bz_sb[:])
                    r_sb = wpool.tile([H, TL], F32, tag="r")
                    nc.scalar.activation(r_sb[:], ar_ps[:], SIG, bias=br_sb[:])
                    rh_sb = wpool.tile([H, TL], F32R, tag="rh")
                    nc.vector.tensor_mul(rh_sb[:], r_sb[:], hT_sb[:])
                    nc.tensor.matmul(
                        ah_ps[:],
                        lhsT=uh_sb[:],
                        rhs=rh_sb[:],
                        start=False,
                        stop=True,
                        skip_group_check=True,
                    )
                    ht_sb = wpool.tile([H, TL], F32, tag="ht")
                    nc.scalar.activation(ht_sb[:], ah_ps[:], TANH, bias=bh_sb[:])

                    d_sb = wpool.tile([H, TL], F32, tag="d")
                    nc.vector.tensor_sub(d_sb[:], ht_sb[:], hT_sb[:])
                    zd_sb = wpool.tile([H, TL], F32, tag="zd")
                    nc.vector.tensor_mul(zd_sb[:], z_sb[:], d_sb[:])
                    hnewT_sb = wpool.tile([H, TL], F32R, tag="hT", bufs=2)
                    nc.vector.tensor_add(hnewT_sb[:], zd_sb[:], hT_sb[:])

                    # -- transpose hnewT -> [TL-chunked, H] payload --
                    if last:
                        hnew_sb = wpool.tile([128, 2 * H], F32, tag="hnew32")
                    else:
                        hnew_sb = wpool.tile([128, 2 * H], F16, tag="hnew16")
                    for half in range(2):
                        tr_ps = ppool.tile([128, H], F32, tag="tr")
                        nc.tensor.transpose(
                            tr_ps[:],
                            hnewT_sb[:, half * 128 : (half + 1) * 128].bitcast(F32),
                            id64[:],
                        )
                        nc.vector.tensor_copy(
                            hnew_sb[:, half * H : (half + 1) * H], tr_ps[:]
                        )

                    if last:
                        nc.sync.dma_start(
                            out=h_out[:].rearrange("(c p) j -> p c j", p=128),
                            in_=hnew_sb[:].rearrange("p (c j) -> p c j", c=2),
                        )
                    else:
                        if "coll" not in ablate:
                            # payload [128, 128] f16: row p = local notes
                            # {p, 128+p} as two 64-col blocks -> one fully
                            # contiguous 256 B descriptor per partition.
                            ag_in = dpool.tile([128, 128], F16, tag="ag_in")
                            nc.sync.dma_start(out=ag_in[:], in_=hnew_sb[:])
                            ag_out = dpool.tile(
                                [M * 128, 128], F16, tag="ag_out",
                                addr_space="Shared",
                            )
                            nc.gpsimd.collective_compute(
                                "AllGather",
                                mybir.AluOpType.bypass,
                                replica_groups=[list(range(M))],
                                ins=[ag_in[:]],
                                outs=[ag_out[:]],
                            )
                            gather_src = ag_out
                        else:
                            gather_src = hid_in
                        if "hreload" not in ablate:
                            h_sb = wpool.tile([128, NCH * H], F16, tag="h", bufs=2)
                            nc.sync.dma_start(
                                out=h_sb[:].rearrange("p (k x) -> p k x", k=8),
                                in_=gather_src[:].rearrange(
                                    "(p k) x -> p k x", p=128
                                ),
                            )
                        hT_sb = hnewT_sb

    return nc


def _host_prep(input, hidden, edge_matrix, ba, wz_wr_wh, uz_ur, uh, input_wzrh, bw):
    """Pre-arrange full inputs into the per-core DMA layouts."""
    x = np.asarray(input, np.float32)[0]  # [T, IN]
    h0 = np.ascontiguousarray(np.asarray(hidden, np.float32)[0])  # [T, H]
    edge = np.asarray(edge_matrix, np.float32)  # [E, T, T]
    ba = np.asarray(ba, np.float32)
    W = np.asarray(wz_wr_wh, np.float32)  # [E, H, 3H]
    uzr = np.ascontiguousarray(np.asarray(uz_ur, np.float32))
    uh_ = np.ascontiguousarray(np.asarray(uh, np.float32))
    win = np.ascontiguousarray(np.asarray(input_wzrh, np.float32))
    bw = np.asarray(bw, np.float32)

    # folded bias: bw + sum_e ba[e] @ W[e]
    btot = bw + np.einsum("eh,ehk->k", ba, W)  # [3H]
    bz = np.ascontiguousarray(btot[:H].reshape(H, 1))
    br = np.ascontiguousarray(btot[H : 2 * H].reshape(H, 1))
    bh = np.ascontiguousarray(btot[2 * H :].reshape(H, 1))

    # edge shards with the permuted source layout (fp16):
    #   esh[m][c_idx*128 + p, e*TL + tl] = edge[e, _S_OF[c_idx, p], m*TL + tl]
    edge_perm = edge[:, _S_OF.reshape(-1), :]  # [E, 2048(perm), T]
    esh = np.ascontiguousarray(
        edge_perm.reshape(E, T, M, TL).transpose(2, 1, 0, 3), dtype=np.float16
    ).reshape(M, T, E * TL)

    # initial h in AG-output layout [8*128, 128] f16:
    #   hid[r, ch*64 + j] = h0[(r//128)*256 + ch*128 + (r%128), j]
    hid = np.ascontiguousarray(
        h0.reshape(M, 2, 128, H).transpose(0, 2, 1, 3).reshape(M * 128, 128),
        dtype=np.float16,
    )

    w_flat = np.ascontiguousarray(
        W.transpose(1, 0, 2), dtype=np.float16
    ).reshape(H, E * 3 * H)

    in_maps = []
    for m in range(M):
        xT = np.ascontiguousarray(x[m * TL : (m + 1) * TL, :].T)
        hT0 = np.ascontiguousarray(h0[m * TL : (m + 1) * TL, :].T)
        in_maps.append(
            {
                "edge_in": esh[m],
                "hid_in": hid,
                "hT0_in": hT0,
                "xT_in": xT,
                "w_in": w_flat,
                "uzr_in": uzr,
                "uh_in": uh_,
                "win_in": win,
                "bz_in": bz,
                "br_in": br,
                "bh_in": bh,
            }
        )
    return in_maps


_NC_CACHE: dict = {}


def _get_nc(iteration: int, reps: int = 1, ablate: frozenset = frozenset()) -> bass.Bass:
    key = (iteration, reps, ablate)
    if key not in _NC_CACHE:
        _NC_CACHE[key] = build(iteration, reps=reps, ablate=ablate)
    return _NC_CACHE[key]


def kernel(
    input,
    hidden,
    edge_matrix,
    ba,
    wz_wr_wh,
    uz_ur,
    uh,
    input_wzrh,
    bw,
    iteration,
):
    iteration = int(iteration)
    if iteration <= 0:
        return np.asarray(hidden, np.float32).copy()

    nc = _get_nc(iteration)
    in_maps = _host_prep(
        input, hidden, edge_matrix, ba, wz_wr_wh, uz_ur, uh, input_wzrh, bw
    )
    res = run_bass_kernel_spmd(nc, in_maps, list(range(M)))
    out = np.concatenate([res.results[m]["h_out"] for m in range(M)], axis=0)
    return out[None]


# revision 12
# speedup vs baseline: 10.2683x; 1.6594x over previous
"""Trainium2 Bass kernel for GatedGraphXBias (gnn_message_passing).

Reference math per iteration (T=2048 notes, E=12 edge types, H=64):
    act[e]  = edge[e].T @ h                      # [T, H]
    a       = sum_e (act[e] + ba[e]) @ W[e] + bw # [T, 3H] -> az|ar|ah
    a      += x @ Win                            # hoisted input projection
    z       = sigmoid(az + h @ Uz)
    r       = sigmoid(ar + h @ Ur)
    h~      = tanh(ah + (r*h) @ Uh)
    h       = (1-z)*h + z*h~

Sharding: sequence-parallel over the note dim T across 8 cores (256 notes
each).  Each core keeps its edge shard [12, 2048, 256] resident in SBUF as
fp16 (12.6 MiB) and the full h replicated as fp16 matmul weights; per
iteration the updated h shard is AllGather'd (fp16, 32 KiB payload).

The big matmuls (edge contraction mm1 and the per-edge-type W projection
mm2) run in fp16 — full 16-bit PE rate, 2x over fp32r, final rel err
~2e-3 (validated against an fp32 numpy sim).  Gate math stays fp32.

The full-h weight copy uses a permuted source layout (source note
s = f(partition, chunk) chosen so the per-iteration h reload from the
AllGather output is 128 contiguous 2 KiB descriptors instead of 2048
strided 256 B ones).  mm2 for pair p-1 is emitted after mm1 of pair p so
the PE never stalls on the PSUM->SBUF act copy.
"""

import sys

sys.path.insert(0, "/opt/trn_rl_repo")

import numpy as np
import concourse.bass as bass
import concourse.mybir as mybir
import concourse.tile as tile
from concourse.bass_utils import run_bass_kernel_spmd
from concourse.masks import make_identity
from concourse.vector_clock import ScopedClock

E, T, H, IN = 12, 2048, 64, 128
M = 8  # cores
TL = T // M  # 256 local notes per core
NCH = T // 128  # 16 contraction chunks of 128 source notes
NPAIR = E // 2  # edge types processed two at a time (n=512 matmuls)
F32 = mybir.dt.float32
F32R = mybir.dt.float32r
F16 = mybir.dt.float16
SIG = mybir.ActivationFunctionType.Sigmoid
TANH = mybir.ActivationFunctionType.Tanh


def _source_perm() -> np.ndarray:
    """s_of[c_idx, p] = global source note held at (chunk c_idx, partition p).

    Chosen so that the h reload from the AllGather output [8*128, 128]
    (row r = rank*128 + p2 holding h rows rank*256 + {0,1}*128 + p2 as two
    64-col blocks) is contiguous per SBUF partition: partition p reads
    DRAM rows p*8 .. p*8+7 (2 KiB).  Chunk c_idx = k*2 + ch maps to
    ag-row r = p*8 + k, giving s = (r//128)*256 + ch*128 + (r%128).
    """
    p = np.arange(128)[None, :]
    c_idx = np.arange(NCH)[:, None]
    k, ch = c_idx // 2, c_idx % 2
    r = p * 8 + k
    return (r // 128) * 256 + ch * 128 + (r % 128)


_S_OF = _source_perm()  # [16, 128]


class SplitDrainTileContext(tile.TileContext):
    """TileContext that limits every instruction to a single sync wait.

    This walrus build rejects >1 sync wait command on an instruction
    (setupSyncWait: "Too many sync wait commands"), so extra waits are
    peeled onto standalone same-engine NoOps emitted just before the
    instruction — semantically identical (the engine stream waits
    sequentially at the same program point)."""

    def _commit_instruction(self, inst, lazy_reg_writes: bool = True):
        si = getattr(inst, "sync_info", None)
        if si is not None and len(si.on_wait) > 1:
            waits = list(si.on_wait)
            inst.sync_info = mybir.SyncInfo(
                on_wait=[waits[-1]], on_update=list(si.on_update)
            )
            for w in waits[:-1]:
                nop = mybir.InstNoOp(
                    name=f"splitwait-{self.nc.next_id()}",
                    sync_info=mybir.SyncInfo(on_wait=[w], on_update=[]),
                    bass_nofuse=True,
                    engine=inst.engine,
                )
                super()._commit_instruction(nop, lazy_reg_writes=False)
        super()._commit_instruction(inst, lazy_reg_writes)

    def _drain_and_barrier(self, tick_clock, wait_clock):
        drain_inst = self.nc.sync.drain()
        wait_clock.add_sem_waits(
            drain_inst.ins, ScopedClock({None: tick_clock.global_clock})
        )
        si = drain_inst.ins.sync_info
        waits = list(si.on_wait) if si is not None else []
        upds = list(si.on_update) if si is not None else []
        if len(waits) > 1:
            drain_inst.ins.sync_info = mybir.SyncInfo(on_wait=waits[:1], on_update=upds)
            for w in waits[1:]:
                nop = self.nc.sync.nop(nofuse=True, hint="split_drain_waits")
                nop.ins.sync_info = mybir.SyncInfo(on_wait=[w], on_update=[])

        self.nc.all_engine_barrier()
        assert self.sems is not None
        popped = self.nc._tile_sem_poison_stack.pop()
        assert popped is self._sem_poison
        self.nc.clear_and_free_semaphores(list(self.sems.allocated().values()))
        self.nc.all_engine_barrier()


def build(iteration: int, reps: int = 1, ablate: frozenset = frozenset()) -> bass.Bass:
    # dynamic_dma_scratch_size: SWDGE (gpsimd) descriptor carveout in SBUF.
    # All our DMAs go through the SP/ACT hardware DGE queues, so shrink it
    # from the default 16 KiB/partition.
    nc = bass.Bass(
        "TRN2",
        target_bir_lowering=False,
        debug=False,
        num_devices=M,
        dynamic_dma_scratch_size=2048,
    )

    # Per-core inputs (host pre-arranged into DMA-friendly layouts):
    #   edge_in : [T, E*TL] f16  row = c_idx*128+p (permuted source note
    #             _S_OF[c_idx, p]), col = e*TL + t_local
    #   hid_in  : [8*128, 128] f16  full initial hidden in AG-output layout
    #   hT0_in  : [H, TL]    local initial hidden, transposed (f32)
    #   xT_in   : [IN, TL]   local input features, transposed (f32)
    #   w_in    : [2H, NPAIR*3H] f16  W[2p],W[2p+1] K-stacked lhsT blocks
    #   uzr_in  : [H, 2H]    f32r
    #   uh_in   : [H, H]     f32r
    #   win_in  : [IN, 3H]   f32r
    #   bz/br/bh: [H, 1]     folded biases (bw + sum_e ba[e] @ W[e])
    edge_in = nc.declare_dram_parameter("edge_in", [T, E * TL], F16, isOutput=False)
    hid_in = nc.declare_dram_parameter("hid_in", [M * 128, 128], F16, isOutput=False)
    hT0_in = nc.declare_dram_parameter("hT0_in", [H, TL], F32R, isOutput=False)
    xT_in = nc.declare_dram_parameter("xT_in", [IN, TL], F32R, isOutput=False)
    w_in = nc.declare_dram_parameter(
        "w_in", [2 * H, NPAIR * 3 * H], F16, isOutput=False
    )
    uzr_in = nc.declare_dram_parameter("uzr_in", [H, 2 * H], F32R, isOutput=False)
    uh_in = nc.declare_dram_parameter("uh_in", [H, H], F32R, isOutput=False)
    win_in = nc.declare_dram_parameter("win_in", [IN, 3 * H], F32R, isOutput=False)
    bz_in = nc.declare_dram_parameter("bz_in", [H, 1], F32, isOutput=False)
    br_in = nc.declare_dram_parameter("br_in", [H, 1], F32, isOutput=False)
    bh_in = nc.declare_dram_parameter("bh_in", [H, 1], F32, isOutput=False)
    h_out = nc.declare_dram_parameter("h_out", [TL, H], F32, isOutput=True)

    with SplitDrainTileContext(nc) as tc:
        with (
            tc.tile_pool(name="edge", bufs=1) as edge_pool,
            tc.tile_pool(name="const", bufs=1) as cpool,
            tc.tile_pool(name="work", bufs=1) as wpool,
            tc.tile_pool(name="psum", bufs=1, space="PSUM") as ppool,
            tc.tile_pool(name="dram", bufs=4, space="DRAM") as dpool,
        ):
            # ---- constants / weights (loaded once) ----
            w_sb = cpool.tile([2 * H, NPAIR * 3 * H], F16)
            nc.sync.dma_start(out=w_sb[:], in_=w_in[:])
            uzr_sb = cpool.tile([H, 2 * H], F32R)
            nc.sync.dma_start(out=uzr_sb[:], in_=uzr_in[:])
            uh_sb = cpool.tile([H, H], F32R)
            nc.sync.dma_start(out=uh_sb[:], in_=uh_in[:])
            win_sb = cpool.tile([IN, 3 * H], F32R)
            nc.sync.dma_start(out=win_sb[:], in_=win_in[:])
            xT_sb = cpool.tile([IN, TL], F32R)
            nc.sync.dma_start(out=xT_sb[:], in_=xT_in[:])
            bz_sb = cpool.tile([H, 1], F32)
            nc.sync.dma_start(out=bz_sb[:], in_=bz_in[:])
            br_sb = cpool.tile([H, 1], F32)
            nc.sync.dma_start(out=br_sb[:], in_=br_in[:])
            bh_sb = cpool.tile([H, 1], F32)
            nc.sync.dma_start(out=bh_sb[:], in_=bh_in[:])
            id64 = cpool.tile([H, H], F32)
            make_identity(nc, id64[:])

            for rep in range(reps):
                # ---- resident edge shard: 16 chunk tiles [128, E*TL] f16 ----
                edge_sb = []
                for c in range(NCH):
                    et = edge_pool.tile(
                        [128, E * TL], F16, name=f"edge_c{c}", tag=f"edge_c{c}",
                        bufs=2,
                    )
                    nc.sync.dma_start(
                        out=et[:], in_=edge_in[c * 128 : (c + 1) * 128, :]
                    )
                    edge_sb.append(et)

                # ---- h state: full h as f16 weights + local hT (f32) ----
                # h_sb[p, c_idx*64 + j] = h[_S_OF[c_idx, p], j]; the load is
                # 8 contiguous 256 B blocks per partition (2 KiB).
                h_sb = wpool.tile([128, NCH * H], F16, name="h", tag="h", bufs=2)
                nc.sync.dma_start(
                    out=h_sb[:].rearrange("p (k x) -> p k x", k=8),
                    in_=hid_in[:].rearrange("(p k) x -> p k x", p=128),
                )
                hT_sb = wpool.tile([H, TL], F32R, name="hT", tag="hT", bufs=2)
                nc.sync.dma_start(out=hT_sb[:], in_=hT0_in[:])

                for it in range(iteration):
                    last = it == iteration - 1

                    # Three PSUM accumulation groups (all [H, TL] base
                    # partition 0): az, ar, ah.  Each starts with the folded
                    # input projection, absorbs the U-gate matmul, then the
                    # 12 per-edge-type W matmuls.
                    az_ps = ppool.tile([H, TL], F32, tag="az")
                    ar_ps = ppool.tile([H, TL], F32, tag="ar")
                    ah_ps = ppool.tile([H, TL], F32, tag="ah")
                    for g, ps in enumerate((az_ps, ar_ps, ah_ps)):
                        nc.tensor.matmul(
                            ps[:],
                            lhsT=win_sb[:, g * H : (g + 1) * H],
                            rhs=xT_sb[:],
                            start=True,
                            stop=False,
                            skip_group_check=True,
                        )
                    for g, ps in enumerate((az_ps, ar_ps)):
                        nc.tensor.matmul(
                            ps[:],
                            lhsT=uzr_sb[:, g * H : (g + 1) * H],
                            rhs=hT_sb[:],
                            start=False,
                            stop="mm2" in ablate or "mm1" in ablate,
                            skip_group_check=True,
                        )

                    # -- mm1: pair-stacked actT [2H, TL] (e-even rows 0:64,
                    # e-odd rows 64:128) so mm2 runs as K=128 matmuls (half
                    # the count, full partition rate); mm2 for pair p-1 is
                    # emitted after mm1 of pair p so the PE never waits on
                    # the PSUM->SBUF act copy --
                    def emit_mm1(pair):
                        act_ps = ppool.tile([2 * H, TL], F32, tag="actT", bufs=4)
                        for c in range(NCH):
                            for half in range(2):
                                e = pair * 2 + half
                                nc.tensor.matmul(
                                    act_ps[half * H : (half + 1) * H, :],
                                    lhsT=h_sb[:, c * H : (c + 1) * H],
                                    rhs=edge_sb[c][:, e * TL : (e + 1) * TL],
                                    start=(c == 0),
                                    stop=(c == NCH - 1),
                                    skip_group_check=True,
                                )
                        act_sb = wpool.tile(
                            [2 * H, TL], F16, name="act", tag="act", bufs=2
                        )
                        nc.vector.tensor_copy(act_sb[:], act_ps[:])
                        return act_sb

                    def emit_mm2(pair, act_sb):
                        if "mm2" in ablate:
                            return
                        for g, ps in enumerate((az_ps, ar_ps, ah_ps)):
                            nc.tensor.matmul(
                                ps[:],
                                lhsT=w_sb[
                                    :,
                                    pair * 3 * H + g * H : pair * 3 * H
                                    + (g + 1) * H,
                                ],
                                rhs=act_sb[:],
                                start=False,
                                stop=(pair == NPAIR - 1 and g != 2),
                                skip_group_check=True,
                            )

                    if "mm1" not in ablate:
                        prev = None
                        for pair in range(NPAIR):
                            act_sb = emit_mm1(pair)
                            if prev is not None:
                                emit_mm2(pair - 1, prev)
                            prev = act_sb
                        emit_mm2(NPAIR - 1, prev)

                    # -- gates --
                    z_sb = wpool.tile([H, TL], F32, tag="z")
                    nc.scalar.activation(z_sb[:], az_ps[:], SIG, bias=bz_sb[:])
                    r_sb = wpool.tile([H, TL], F32, tag="r")
                    nc.scalar.activation(r_sb[:], ar_ps[:], SIG, bias=br_sb[:])
                    rh_sb = wpool.tile([H, TL], F32R, tag="rh")
                    nc.vector.tensor_mul(rh_sb[:], r_sb[:], hT_sb[:])
                    nc.tensor.matmul(
                        ah_ps[:],
                        lhsT=uh_sb[:],
                        rhs=rh_sb[:],
                        start=False,
                        stop=True,
                        skip_group_check=True,
                    )
                    ht_sb = wpool.tile([H, TL], F32, tag="ht")
                    nc.scalar.activation(ht_sb[:], ah_ps[:], TANH, bias=bh_sb[:])

                    d_sb = wpool.tile([H, TL], F32, tag="d")
                    nc.vector.tensor_sub(d_sb[:], ht_sb[:], hT_sb[:])
                    zd_sb = wpool.tile([H, TL], F32, tag="zd")
                    nc.vector.tensor_mul(zd_sb[:], z_sb[:], d_sb[:])
                    hnewT_sb = wpool.tile([H, TL], F32R, tag="hT", bufs=2)
                    nc.vector.tensor_add(hnewT_sb[:], zd_sb[:], hT_sb[:])

                    # -- transpose hnewT -> [TL-chunked, H] payload --
                    if last:
                        hnew_sb = wpool.tile([128, 2 * H], F32, tag="hnew32")
                    else:
                        hnew_sb = wpool.tile([128, 2 * H], F16, tag="hnew16")
                    for half in range(2):
                        tr_ps = ppool.tile([128, H], F32, tag="tr")
                        nc.tensor.transpose(
                            tr_ps[:],
                            hnewT_sb[:, half * 128 : (half + 1) * 128].bitcast(F32),
                            id64[:],
                        )
                        nc.vector.tensor_copy(
                            hnew_sb[:, half * H : (half + 1) * H], tr_ps[:]
                        )

                    if last:
                        nc.sync.dma_start(
                            out=h_out[:].rearrange("(c p) j -> p c j", p=128),
                            in_=hnew_sb[:].rearrange("p (c j) -> p c j", c=2),
                        )
                    else:
                        if "coll" not in ablate:
                            # payload [128, 128] f16: row p = local notes
                            # {p, 128+p} as two 64-col blocks -> one fully
                            # contiguous 256 B descriptor per partition.
                            ag_in = dpool.tile([128, 128], F16, tag="ag_in")
                            nc.sync.dma_start(out=ag_in[:], in_=hnew_sb[:])
                            ag_out = dpool.tile(
                                [M * 128, 128], F16, tag="ag_out",
                                addr_space="Shared",
                            )
                            nc.gpsimd.collective_compute(
                                "AllGather",
                                mybir.AluOpType.bypass,
                                replica_groups=[list(range(M))],
                                ins=[ag_in[:]],
                                outs=[ag_out[:]],
                            )
                            gather_src = ag_out
                        else:
                            gather_src = hid_in
                        if "hreload" not in ablate:
                            h_sb = wpool.tile([128, NCH * H], F16, tag="h", bufs=2)
                            nc.sync.dma_start(
                                out=h_sb[:].rearrange("p (k x) -> p k x", k=8),
                                in_=gather_src[:].rearrange(
                                    "(p k) x -> p k x", p=128
                                ),
                            )
                        hT_sb = hnewT_sb

    return nc


def _host_prep(input, hidden, edge_matrix, ba, wz_wr_wh, uz_ur, uh, input_wzrh, bw):
    """Pre-arrange full inputs into the per-core DMA layouts."""
    x = np.asarray(input, np.float32)[0]  # [T, IN]
    h0 = np.ascontiguousarray(np.asarray(hidden, np.float32)[0])  # [T, H]
    edge = np.asarray(edge_matrix, np.float32)  # [E, T, T]
    ba = np.asarray(ba, np.float32)
    W = np.asarray(wz_wr_wh, np.float32)  # [E, H, 3H]
    uzr = np.ascontiguousarray(np.asarray(uz_ur, np.float32))
    uh_ = np.ascontiguousarray(np.asarray(uh, np.float32))
    win = np.ascontiguousarray(np.asarray(input_wzrh, np.float32))
    bw = np.asarray(bw, np.float32)

    # folded bias: bw + sum_e ba[e] @ W[e]
    btot = bw + np.einsum("eh,ehk->k", ba, W)  # [3H]
    bz = np.ascontiguousarray(btot[:H].reshape(H, 1))
    br = np.ascontiguousarray(btot[H : 2 * H].reshape(H, 1))
    bh = np.ascontiguousarray(btot[2 * H :].reshape(H, 1))

    # edge shards with the permuted source layout (fp16):
    #   esh[m][c_idx*128 + p, e*TL + tl] = edge[e, _S_OF[c_idx, p], m*TL + tl]
    edge_perm = edge[:, _S_OF.reshape(-1), :]  # [E, 2048(perm), T]
    esh = np.ascontiguousarray(
        edge_perm.reshape(E, T, M, TL).transpose(2, 1, 0, 3), dtype=np.float16
    ).reshape(M, T, E * TL)

    # initial h in AG-output layout [8*128, 128] f16:
    #   hid[r, ch*64 + j] = h0[(r//128)*256 + ch*128 + (r%128), j]
    hid = np.ascontiguousarray(
        h0.reshape(M, 2, 128, H).transpose(0, 2, 1, 3).reshape(M * 128, 128),
        dtype=np.float16,
    )

    # K-stacked pair weights: w_flat[i2*H + j, p*3H + gk] = W[2p+i2][j, gk]
    w_flat = np.ascontiguousarray(
        W.reshape(NPAIR, 2, H, 3 * H).transpose(1, 2, 0, 3), dtype=np.float16
    ).reshape(2 * H, NPAIR * 3 * H)

    in_maps = []
    for m in range(M):
        xT = np.ascontiguousarray(x[m * TL : (m + 1) * TL, :].T)
        hT0 = np.ascontiguousarray(h0[m * TL : (m + 1) * TL, :].T)
        in_maps.append(
            {
                "edge_in": esh[m],
                "hid_in": hid,
                "hT0_in": hT0,
                "xT_in": xT,
                "w_in": w_flat,
                "uzr_in": uzr,
                "uh_in": uh_,
                "win_in": win,
                "bz_in": bz,
                "br_in": br,
                "bh_in": bh,
            }
        )
    return in_maps


_NC_CACHE: dict = {}


def _get_nc(iteration: int, reps: int = 1, ablate: frozenset = frozenset()) -> bass.Bass:
    key = (iteration, reps, ablate)
    if key not in _NC_CACHE:
        _NC_CACHE[key] = build(iteration, reps=reps, ablate=ablate)
    return _NC_CACHE[key]


def kernel(
    input,
    hidden,
    edge_matrix,
    ba,
    wz_wr_wh,
    uz_ur,
    uh,
    input_wzrh,
    bw,
    iteration,
):
    iteration = int(iteration)
    if iteration <= 0:
        return np.asarray(hidden, np.float32).copy()

    nc = _get_nc(iteration)
    in_maps = _host_prep(
        input, hidden, edge_matrix, ba, wz_wr_wh, uz_ur, uh, input_wzrh, bw
    )
    res = run_bass_kernel_spmd(nc, in_maps, list(range(M)))
    out = np.concatenate([res.results[m]["h_out"] for m in range(M)], axis=0)
    return out[None]
